# revision 28
# baseline (speedup 1.0000x reference)
"""Trainium2 Bass kernel for a dense transformer block (RoPE attention + SwiGLU).

Sharding (8 NeuronCores, Megatron-style):
  - QKV + attention: tensor-parallel over heads (2 heads/core, both batches).
  - Two AllToAlls (one per batch) reshard attention output from head-sharded
    to token-sharded (each core owns tpc/2 tokens of each batch).
  - proj + SwiGLU MLP: token-sharded, fully local, split into two halves so
    the first half overlaps batch-1 attention and the second hides A2A#1.
Engine queues are in-order, so the program interleaves QKV(b1) into the PE
gaps of attention(b0), and proj(half 0) into attention(b1).
All matmuls run in bf16 with fp32 PSUM accumulation.  silu is computed as
0.5*x*(1+tanh(x/2)) so the scalar engine stays on one activation table set
(exp+tanh); the 0.5 is folded into w2 on the host.
"""

import functools
import numpy as np
import ml_dtypes

B, T, C, H, D = 2, 2048, 1024, 16, 64
HID = 4 * C
NCORES = 8
HPC = H // NCORES          # heads per core
PACK_SCORES = True         # row-pack the two heads' score matmuls


def _build_program(b, t):
    import concourse.bacc as bacc
    import concourse.mybir as mybir
    import concourse.tile as tile
    import concourse.masks as masks
    from contextlib import ExitStack

    fp32 = mybir.dt.float32
    bf16 = mybir.dt.bfloat16
    Act = mybir.ActivationFunctionType
    Alu = mybir.AluOpType

    tok = b * t                    # all tokens (b-major)
    tpc = tok // NCORES            # tokens per core (proj/MLP/out)
    half = tpc // b                # tokens per core per batch
    m_qkv = 3 * HPC * D            # q, k, v local cols
    kt_tiles = t // 128            # 128-token key tiles per (b,h) unit
    qt_chunk = min(512, t)
    qcs = t // qt_chunk            # q chunks (= QKV n-chunks) per batch
    kt_pc = qt_chunk // 128        # kt tiles per QKV chunk
    ck = C // 128                  # C chunks (8)
    mh_tiles = HID // 128          # hidden chunks (32)
    hg = 8                         # hidden chunks per weight-stream group
    ngr = mh_tiles // hg           # weight groups per half
    tbs = min(128, half)           # token block size for w3
    ntb = half // tbs              # token blocks per half for w3
    scale = float(D) ** -0.5

    nc = bacc.Bacc("TRN2", target_bir_lowering=False, debug=False,
                   num_devices=NCORES)

    # ---- DRAM I/O ----
    xT_d = nc.dram_tensor("xT", [C, tok], bf16, kind="ExternalInput")
    wqkvT_d = nc.dram_tensor("wqkvT", [C, m_qkv], bf16, kind="ExternalInput")
    bqkv_d = nc.dram_tensor("bqkv2d", [128, 3], fp32, kind="ExternalInput")
    cos_d = nc.dram_tensor("cosd", [128, tok], bf16, kind="ExternalInput")
    sin_d = nc.dram_tensor("sind", [128, tok], bf16, kind="ExternalInput")
    wprojT_d = nc.dram_tensor("wprojT", [C, C], bf16, kind="ExternalInput")
    bproj_d = nc.dram_tensor("bproj2d", [128, ck], fp32, kind="ExternalInput")
    w1T_d = nc.dram_tensor("w1T", [C, HID], bf16, kind="ExternalInput")
    w2Ts_d = nc.dram_tensor("w2Ts", [C, HID], bf16, kind="ExternalInput")
    w3T_d = nc.dram_tensor("w3T", [HID, C], bf16, kind="ExternalInput")
    b1h_d = nc.dram_tensor("b1h2d", [128, mh_tiles], fp32, kind="ExternalInput")
    b2s_d = nc.dram_tensor("b2s2d", [128, mh_tiles], fp32, kind="ExternalInput")
    b3row_d = nc.dram_tensor("b3row", [1, C], bf16, kind="ExternalInput")
    y_d = nc.dram_tensor("y_loc", [tpc, C], fp32, kind="ExternalOutput")

    with tile.TileContext(nc) as tc:
        es = ExitStack()
        # ---- constants / biases (live whole kernel) ----
        consts = es.enter_context(tc.tile_pool(name="consts", bufs=1))
        ident = consts.tile([128, 128], bf16, name="ident")
        masks.make_identity(nc, ident[:])
        ones1 = consts.tile([65, 64], bf16, name="ones1")
        nc.vector.memset(ones1[:], 1.0)
        onescol = consts.tile([1, 128], bf16, name="onescol")
        nc.vector.memset(onescol[:], 1.0)
        lnscr = consts.tile([1, 8], fp32, name="lnscr")
        b3row = consts.tile([1, C], bf16, name="b3row")
        nc.sync.dma_start(out=b3row[:], in_=b3row_d[:, :])
        bqkv_sb = consts.tile([128, 3], fp32, name="bqkv_sb")
        nc.sync.dma_start(out=bqkv_sb[:], in_=bqkv_d[:, :])
        bproj_sb = consts.tile([128, ck], fp32, name="bproj_sb")
        nc.sync.dma_start(out=bproj_sb[:], in_=bproj_d[:, :])
        b1h_sb = consts.tile([128, mh_tiles], fp32, name="b1h_sb")
        nc.sync.dma_start(out=b1h_sb[:], in_=b1h_d[:, :])
        b2s_sb = consts.tile([128, mh_tiles], fp32, name="b2s_sb")
        nc.sync.dma_start(out=b2s_sb[:], in_=b2s_d[:, :])

        # ---- DRAM bounce buffers for the two A2As ----
        dram = es.enter_context(tc.tile_pool(name="dramp", bufs=1,
                                             space="DRAM"))
        a2a_in = [dram.tile([NCORES * 128, half], bf16, name=f"a2a_in{i}")
                  for i in range(b)]
        a2a_out = [dram.tile([NCORES * 128, half], bf16, name=f"a2a_out{i}")
                   for i in range(b)]

        # ---- attention pools (live whole kernel; w3 reuses their PSUM) ----
        attn_pool = es.enter_context(tc.tile_pool(name="attn", bufs=1))
        qr = attn_pool.tile([128, tok], bf16, name="qr")
        kr = attn_pool.tile([128, tok], bf16, name="kr")
        vaug_cols = 65 * kt_tiles * b * HPC
        v_aug = attn_pool.tile([128, vaug_cols], bf16, name="v_aug")
        nc.vector.memset(v_aug[:], 1.0)
        outT0 = attn_pool.tile([64, tok], bf16, name="outT0")
        outT1 = attn_pool.tile([64, tok], bf16, name="outT1")
        ps_s = es.enter_context(
            tc.tile_pool(name="ps_s", bufs=2, space="PSUM"))
        ps_o = es.enter_context(
            tc.tile_pool(name="ps_o", bufs=4, space="PSUM"))
        expp = es.enter_context(tc.tile_pool(name="expp", bufs=3))
        smp = es.enter_context(tc.tile_pool(name="smp", bufs=2))

        # ---- QKV-phase pools (closed after QKV/rope/v of both batches) ----
        es_q = ExitStack()
        wq_pool = es_q.enter_context(tc.tile_pool(name="wq", bufs=1))
        rope_scr = es_q.enter_context(tc.tile_pool(name="ropescr", bufs=2))
        xt_pool = [None, None]
        qraw = [None, None]        # per-batch raw qkv (pool, tiles)

        wq_all = wq_pool.tile([128, ck * m_qkv], bf16, name="wq_all")
        for q in range(2):
            kc0 = q * (ck // 2)
            nc.sync.dma_start(
                out=wq_all[:, kc0 * m_qkv:(kc0 + ck // 2) * m_qkv].rearrange(
                    "p (k c) -> p k c", k=ck // 2),
                in_=wqkvT_d[128 * kc0:128 * (kc0 + ck // 2), :].rearrange(
                    "(k p) c -> p k c", p=128))
        wq_sb = [wq_all[:, kc * m_qkv:(kc + 1) * m_qkv] for kc in range(ck)]
        cos_sb = wq_pool.tile([128, tok], bf16, name="cos_sb")
        sin_sb = wq_pool.tile([128, tok], bf16, name="sin_sb")

        xt_sb = [[None] * ck, [None] * ck]

        def emit_x_loads(bi):
            pool = tc.alloc_tile_pool(name=f"xt{bi}", bufs=1)
            xt_pool[bi] = pool
            for kc in range(ck):
                xt_kc = pool.tile([128, t], bf16, name=f"xt{bi}_{kc}",
                                  tag=f"xt{kc}")
                for n in range(qcs):
                    nc.sync.dma_start(
                        out=xt_kc[:, n * qt_chunk:(n + 1) * qt_chunk],
                        in_=xT_d[128 * kc:128 * kc + 128,
                                 bi * t + n * qt_chunk:
                                 bi * t + (n + 1) * qt_chunk])
                xt_sb[bi][kc] = xt_kc

        def open_qraw(bi):
            pool = tc.alloc_tile_pool(name=f"qraw{bi}", bufs=1)
            tiles = [pool.tile([128, t], bf16, name=f"{nm}_{bi}")
                     for nm in ("q", "k", "v", "qsw", "ksw")]
            qraw[bi] = (pool, tiles)

        def emit_qkv_slice(bi, n, mi):
            """One [128 out-rows, qt_chunk tokens] slice of the QKV GEMM."""
            _, dest = qraw[bi]
            c0 = n * qt_chunk
            ps = ps_s.tile([128, qt_chunk], fp32, name=f"psq{bi}{n}{mi}",
                           tag="pss")
            for kc in range(ck):
                nc.tensor.matmul(
                    ps[:], wq_sb[kc][:, 128 * mi:128 * mi + 128],
                    xt_sb[bi][kc][:, c0:c0 + qt_chunk],
                    start=(kc == 0), stop=(kc == ck - 1))
            nc.vector.tensor_scalar_add(
                dest[mi][:, c0:c0 + qt_chunk], ps[:], bqkv_sb[:, mi:mi + 1])

        def emit_swaps(bi):
            """qsw/ksw = q/k with each head's two 32-row halves swapped."""
            _, d = qraw[bi]
            for src_t, dst_t in ((d[0], d[3]), (d[1], d[4])):
                for h in range(HPC):
                    for half32 in range(2):
                        s0 = 64 * h + 32 * half32
                        d0 = 64 * h + 32 * (1 - half32)
                        nc.sync.dma_start(
                            out=dst_t[d0:d0 + 32, :],
                            in_=src_t[s0:s0 + 32, :])

        def emit_rope(bi):
            _, d = qraw[bi]
            q_bf, k_bf, _, qsw_bf, ksw_bf = d
            cs = cos_sb[:, bi * t:bi * t + t]
            sn = sin_sb[:, bi * t:bi * t + t]
            for u_src, u_sw, u_dst in ((q_bf, qsw_bf, qr), (k_bf, ksw_bf, kr)):
                ta = rope_scr.tile([128, t], bf16, name="ta", tag="rs")
                nc.vector.tensor_mul(ta[:], u_src[:], cs)
                tb = rope_scr.tile([128, t], bf16, name="tb", tag="rs")
                nc.vector.tensor_mul(tb[:], u_sw[:], sn)
                nc.vector.tensor_add(u_dst[:, bi * t:bi * t + t], ta[:], tb[:])

        def emit_vtr(bi, kt):
            """Transpose v[:, kt] into v_aug for both heads."""
            _, d = qraw[bi]
            v_bf = d[2]
            for h in range(HPC):
                u = bi * HPC + h
                base = u * 65 * kt_tiles
                pst = ps_s.tile([128, 64], bf16, name=f"pst{u}_{kt}",
                                tag="pss")
                nc.tensor.transpose(
                    pst[:],
                    v_bf[64 * h:64 * h + 64, 128 * kt:128 * kt + 128],
                    ident[64 * h:64 * h + 64, 64 * h:64 * h + 64])
                nc.vector.tensor_copy(
                    v_aug[:, base + 65 * kt:base + 65 * kt + 64], pst[:])

        # ---------- deque-based interleave machinery ----------
        pending = []               # list of zero-arg emitters

        def pump(k):
            for _ in range(min(k, len(pending))):
                pending.pop(0)()

        def emit_unit_pair(bi, qlist):
            """Attention for both heads of (batch bi, q-chunks qlist).

            Both heads' scores for one kt land in one 2-bank psum tile
            (row-packed matmuls writing the two column halves), so a single
            FD=2*qt_chunk exp evacuates the pair.  Two q-chunks are kept in
            flight to hide the PE<->ACT semaphore latency."""
            q0 = {qc: bi * t + qc * qt_chunk for qc in qlist}
            pso = {(qc, h): ps_o.tile([65, qt_chunk], fp32,
                                      name=f"pso{bi}{qc}{h}", tag="pso")
                   for qc in qlist for h in range(HPC)}
            # dummy Ln keeps this group's exps on the ln+exp table set
            nc.scalar.activation(lnscr[:, :], ones1[0:1, 0:8], Act.Ln)
            for kt in range(kt_tiles):
                pss = {}
                for qc in qlist:
                    p = ps_s.tile([128, 2 * qt_chunk], fp32,
                                  name=f"pss{bi}{qc}{kt}", tag="pss")
                    for h in range(HPC):
                        tp = (64 * h, 0) if PACK_SCORES else None
                        nc.tensor.matmul(
                            p[:, h * qt_chunk:(h + 1) * qt_chunk],
                            kr[64 * h:64 * h + 64,
                               bi * t + 128 * kt:bi * t + 128 * kt + 128],
                            qr[64 * h:64 * h + 64,
                               q0[qc]:q0[qc] + qt_chunk],
                            start=True, stop=True, tile_position=tp)
                    pss[qc] = p
                exps = {}
                for qc in qlist:
                    e = expp.tile([128, 2 * qt_chunk], bf16,
                                  name=f"exp{bi}{qc}{kt}", tag="e")
                    nc.scalar.activation(e[:], pss[qc][:], Act.Exp,
                                         scale=scale)
                    exps[qc] = e
                for qc in qlist:
                    for h in range(HPC):
                        u = bi * HPC + h
                        vbase = u * 65 * kt_tiles
                        nc.tensor.matmul(
                            pso[(qc, h)][:],
                            v_aug[:, vbase + 65 * kt:vbase + 65 * kt + 65],
                            exps[qc][:, h * qt_chunk:(h + 1) * qt_chunk],
                            start=(kt == 0), stop=(kt == kt_tiles - 1))
                if kt % 4 == 3:
                    pump(3)
            # normalization: 1/s = exp(-ln(s)) on ACT, broadcast via
            # matmul, final mul on DVE
            for qc in qlist:
                for h in range(HPC):
                    lnv = smp.tile([65, qt_chunk], fp32,
                                   name=f"ln{bi}{qc}{h}", tag="lnv", bufs=2)
                    nc.scalar.activation(lnv[64:65, :],
                                         pso[(qc, h)][64:65, :], Act.Ln)
                    rb = smp.tile([65, qt_chunk], bf16,
                                  name=f"rb{bi}{qc}{h}", tag="rb", bufs=2)
                    nc.scalar.activation(rb[64:65, :], lnv[64:65, :],
                                         Act.Exp, scale=-1.0)
                    psb = ps_s.tile([64, qt_chunk], fp32,
                                    name=f"psb{bi}{qc}{h}", tag="pss")
                    nc.tensor.matmul(psb[:], ones1[64:65, :], rb[64:65, :],
                                     start=True, stop=True)
                    bc = smp.tile([64, qt_chunk], fp32,
                                  name=f"bc{bi}{qc}{h}", tag="bc", bufs=2)
                    nc.vector.tensor_copy(bc[:], psb[:])
                    out_h = outT0 if h == 0 else outT1
                    nc.vector.tensor_mul(
                        out_h[:, q0[qc]:q0[qc] + qt_chunk],
                        pso[(qc, h)][0:64, :], bc[:])

        def emit_a2a_in(bi, qc):
            lo = qc * qt_chunk
            hi = lo + qt_chunk
            for c in range(NCORES):
                s0 = c * half
                if s0 < lo or s0 >= hi:
                    continue
                for h, out_h in ((0, outT0), (1, outT1)):
                    nc.sync.dma_start(
                        out=a2a_in[bi][128 * c + 64 * h:
                                       128 * c + 64 * h + 64, :],
                        in_=out_h[:, bi * t + s0:bi * t + s0 + half])

        # ================= program =================
        open_qraw(0)
        emit_x_loads(0)
        for n in range(qcs):
            for mi in range(3):
                emit_qkv_slice(0, n, mi)
        xt_pool[0].release()
        nc.sync.dma_start(out=cos_sb[:], in_=cos_d[:, :])
        nc.sync.dma_start(out=sin_sb[:], in_=sin_d[:, :])
        emit_swaps(0)
        emit_rope(0)
        for kt in range(kt_tiles):
            emit_vtr(0, kt)
        qraw[0][0].release()

        # queue QKV b1 + v transposes b1 + rope b1 as pump items
        open_qraw(1)
        emit_x_loads(1)
        vtr_items = []
        for n in range(qcs):
            for mi in range(3):
                pending.append(
                    (lambda n=n, mi=mi: emit_qkv_slice(1, n, mi)))
            for kt in range(n * kt_pc, (n + 1) * kt_pc):
                vtr_items.append((lambda kt=kt: emit_vtr(1, kt)))
        pending.append(lambda: emit_swaps(1))
        pending.append(lambda: emit_rope(1))
        pending.extend(vtr_items)

        # ---- attention batch 0, QKV b1 interleaved ----
        for qp in range(0, qcs, 2):
            qlist = tuple(range(qp, min(qp + 2, qcs)))
            emit_unit_pair(0, qlist)
            for qc in qlist:
                emit_a2a_in(0, qc)
            pump(6)
        pump(len(pending))
        # QKV pools no longer needed; swap PSUM/SBUF budget to MLP pools
        xt_pool[1].release()
        qraw[1][0].release()
        es_q.close()
        nc.gpsimd.collective_compute(
            "AllToAll", Alu.bypass,
            replica_groups=[list(range(NCORES))],
            ins=[a2a_in[0][:]], outs=[a2a_out[0][:]])

        wg_pool = es.enter_context(tc.tile_pool(name="wg", bufs=2))
        mlp_scr = es.enter_context(tc.tile_pool(name="mlp_scr", bufs=2))
        wp_pool = es.enter_context(tc.tile_pool(name="wpp", bufs=1))
        of_pool = es.enter_context(tc.tile_pool(name="ofp", bufs=1))
        mlp_sb = es.enter_context(tc.tile_pool(name="mlp_sb", bufs=1))
        aT = [mlp_sb.tile([128, ck * half], bf16, name=f"aT{i}")
              for i in range(b)]
        hT = [mlp_sb.tile([128, mh_tiles * half], bf16, name=f"hT{i}")
              for i in range(b)]

        wp_all = wp_pool.tile([128, ck * C], bf16, name="wp_all")
        nc.sync.dma_start(
            out=wp_all[:].rearrange("p (k c) -> p k c", k=ck),
            in_=wprojT_d[:, :].rearrange("(k p) c -> p k c", p=128))
        wp_sb = [wp_all[:, kc * C:(kc + 1) * C] for kc in range(ck)]

        of_sb = [[None] * ck, [None] * ck]

        def emit_of_loads(hf):
            of_all = of_pool.tile([128, ck * half], bf16, name=f"of{hf}",
                                  tag=f"of{hf}")
            nc.sync.dma_start(
                out=of_all[:].rearrange("p (k c) -> p k c", k=ck),
                in_=a2a_out[hf][:, :].rearrange("(k p) c -> p k c", p=128))
            for kc in range(ck):
                of_sb[hf][kc] = of_all[:, kc * half:(kc + 1) * half]

        def emit_proj_slice(hf, m):
            psa = ps_o.tile([128, half], fp32, name=f"psa{hf}{m}", tag="pso")
            for kc in range(ck):
                nc.tensor.matmul(psa[:],
                                 wp_sb[kc][:, 128 * m:128 * m + 128],
                                 of_sb[hf][kc][:],
                                 start=(kc == 0), stop=(kc == ck - 1))
            nc.vector.tensor_scalar_add(
                aT[hf][:, half * m:half * (m + 1)], psa[:],
                bproj_sb[:, m:m + 1])

        wg_sb = {}

        def emit_wg_loads(hf, g):
            gw = hg * 128
            for wname, wd in (("w1", w1T_d), ("w2", w2Ts_d)):
                wt = wg_pool.tile([128, ck * gw], bf16,
                                  name=f"{wname}g{hf}{g}", tag=f"{wname}g")
                for q in range(2):
                    kc0 = q * (ck // 2)
                    nc.sync.dma_start(
                        out=wt[:, kc0 * gw:(kc0 + ck // 2) * gw].rearrange(
                            "p (k c) -> p k c", k=ck // 2),
                        in_=wd[128 * kc0:128 * (kc0 + ck // 2),
                               gw * g:gw * (g + 1)].rearrange(
                            "(k p) c -> p k c", p=128))
                wg_sb[(hf, wname, g)] = [wt[:, kc * gw:(kc + 1) * gw]
                                         for kc in range(ck)]

        def emit_mlp_tile(hf, mh):
            g, ml = mh // hg, mh % hg
            w1g = wg_sb[(hf, "w1", g)]
            w2g = wg_sb[(hf, "w2", g)]
            ps1 = ps_o.tile([128, half], fp32, name=f"ps1_{hf}{mh}",
                            tag="pso")
            for kc in range(ck):
                nc.tensor.matmul(ps1[:],
                                 w1g[kc][:, 128 * ml:128 * ml + 128],
                                 aT[hf][:, half * kc:half * (kc + 1)],
                                 start=(kc == 0), stop=(kc == ck - 1))
            ps2 = ps_o.tile([128, half], fp32, name=f"ps2_{hf}{mh}",
                            tag="pso")
            for kc in range(ck):
                nc.tensor.matmul(ps2[:],
                                 w2g[kc][:, 128 * ml:128 * ml + 128],
                                 aT[hf][:, half * kc:half * (kc + 1)],
                                 start=(kc == 0), stop=(kc == ck - 1))
            th = mlp_scr.tile([128, half], fp32, name=f"th{hf}{mh}", tag="th")
            nc.scalar.activation(th[:], ps1[:], Act.Tanh, scale=0.5,
                                 bias=b1h_sb[:, mh:mh + 1])
            uu = mlp_scr.tile([128, half], fp32, name=f"u{hf}{mh}", tag="u")
            nc.vector.scalar_tensor_tensor(uu[:], th[:], 1.0, ps1[:],
                                           op0=Alu.add, op1=Alu.mult)
            nc.vector.scalar_tensor_tensor(
                hT[hf][:, half * mh:half * (mh + 1)], ps2[:],
                b2s_sb[:, mh:mh + 1], uu[:], op0=Alu.add, op1=Alu.mult)


        def emit_w3_group(hf, yps, khp):
            w3k = w3_pool.tile([128, 2 * C], bf16, name=f"w3k{hf}{khp}",
                               tag="w3k")
            nc.sync.dma_start(
                out=w3k[:].rearrange("p (k c) -> p k c", k=2),
                in_=w3T_d[256 * khp:256 * (khp + 1), :].rearrange(
                    "(k p) c -> p k c", p=128))
            for j in range(2):
                kh = 2 * khp + j
                for tb in range(ntb):
                    hslice = hT[hf][:, half * kh + tbs * tb:
                                    half * kh + tbs * (tb + 1)]
                    for ch in range(C // 512):
                        nc.tensor.matmul(
                            yps[tb][:, 512 * ch:512 * (ch + 1)],
                            hslice, w3k[:, C * j + 512 * ch:
                                        C * j + 512 * (ch + 1)],
                            start=(kh == 0), stop=False)

        def emit_w3_tail(hf, yps):
            for tb in range(ntb):
                ysb = ysb_pool.tile([tbs, C], fp32, name=f"ysb{hf}{tb}",
                                    tag="ysb")
                for ch in range(C // 512):
                    nc.tensor.matmul(
                        yps[tb][:, 512 * ch:512 * (ch + 1)],
                        onescol[0:1, 0:tbs],
                        b3row[0:1, 512 * ch:512 * (ch + 1)],
                        start=False, stop=True)
                    nc.vector.tensor_copy(ysb[:, 512 * ch:512 * (ch + 1)],
                                          yps[tb][:, 512 * ch:512 * (ch + 1)])
                nc.sync.dma_start(
                    out=y_d[half * hf + tbs * tb:half * hf + tbs * (tb + 1),
                            :],
                    in_=ysb[:])

        def emit_mlp_half(hf):
            """MLP tiles with the w3 GEMM interleaved as hT tiles appear."""
            yps = [ps_s.tile([tbs, C], fp32, name=f"yps{hf}{tb}",
                             tag="pss") for tb in range(ntb)]
            for mh in range(mh_tiles):
                if mh % hg == 0:
                    gi = mh // hg
                    if gi + 1 < ngr:
                        emit_wg_loads(hf, gi + 1)
                    elif hf == 0:
                        emit_wg_loads(1, 0)
                emit_mlp_tile(hf, mh)
                if mh % 2 == 1:
                    emit_w3_group(hf, yps, mh // 2)
            emit_w3_tail(hf, yps)

        # ---- attention batch 1, of0+proj half 0 pumped in the last pair ----
        emit_wg_loads(0, 0)
        for qp in range(0, qcs, 2):
            qlist = tuple(range(qp, min(qp + 2, qcs)))
            if qcs > 1 and qp + 2 >= qcs:
                pending.append(lambda: emit_of_loads(0))
                for m in range(ck):
                    pending.append((lambda m=m: emit_proj_slice(0, m)))
            emit_unit_pair(1, qlist)
            for qc in qlist:
                emit_a2a_in(1, qc)
            pump(4)
        pump(len(pending))

        nc.gpsimd.collective_compute(
            "AllToAll", Alu.bypass,
            replica_groups=[list(range(NCORES))],
            ins=[a2a_in[1][:]], outs=[a2a_out[1][:]])
        if qcs == 1:   # small-t sim path: emit of0+proj directly
            emit_of_loads(0)
            for m in range(ck):
                emit_proj_slice(0, m)

        # ---- MLP+w3 half 0 (hides A2A#1), then half 1 ----
        w3_pool = es.enter_context(tc.tile_pool(name="w3p", bufs=3))
        ysb_pool = es.enter_context(tc.tile_pool(name="ysb", bufs=1))
        emit_mlp_half(0)
        emit_of_loads(1)
        for m in range(ck):
            emit_proj_slice(1, m)
        emit_mlp_half(1)
        es.close()

    nc.compile()
    return nc


@functools.lru_cache(maxsize=2)
def _get_program(b, t):
    return _build_program(b, t)


def _prep_inputs(x, w_qkv, b_qkv, w_proj, b_proj, w1, b1, w2, b2, w3, b3,
                 cos, sin, b, t):
    """Build per-core in_maps (host-side sharding / transposes / casts)."""
    bf = ml_dtypes.bfloat16
    tok = b * t
    ck = C // 128
    mh_tiles = HID // 128

    xT = np.ascontiguousarray(x.reshape(tok, C).T).astype(bf)
    # RoPE tables tiled to [128, tok]: rows = 4x the 32 freq rows,
    # cols = b-major tokens.
    cosd = np.tile(cos.T, (4, b)).astype(bf)
    sind = np.tile(np.concatenate([-sin.T, sin.T], axis=0), (2, b)).astype(bf)
    wprojT = np.ascontiguousarray(w_proj.T).astype(bf)
    bproj2d = np.ascontiguousarray(b_proj.reshape(ck, 128).T).astype(np.float32)
    w1T = np.ascontiguousarray(w1.T).astype(bf)
    w2Ts = np.ascontiguousarray(0.5 * w2.T).astype(bf)
    w3T = np.ascontiguousarray(w3.T).astype(bf)
    b1h2d = np.ascontiguousarray(
        (0.5 * b1).reshape(mh_tiles, 128).T).astype(np.float32)
    b2s2d = np.ascontiguousarray(
        (0.5 * b2).reshape(mh_tiles, 128).T).astype(np.float32)
    b3row = np.ascontiguousarray(b3.reshape(1, C)).astype(bf)

    # even/odd RoPE permutation within each head's 64 dims, plus the
    # half-swapped copy used to make RoPE partition-aligned on device
    perm = np.concatenate([np.arange(0, D, 2), np.arange(1, D, 2)])
    perm_sw = np.concatenate([np.arange(1, D, 2), np.arange(0, D, 2)])

    in_maps = []
    for c in range(NCORES):
        rows = []
        brows = []
        secperm = [(0, perm), (1, perm), (2, np.arange(D))]
        for sec, p in secperm:        # q, k, v
            for hh in range(HPC):
                h = HPC * c + hh
                idx = sec * H * D + h * D + p
                rows.append(w_qkv[idx, :])
                brows.append(b_qkv[idx])
        wql = np.concatenate(rows, axis=0)           # [384, C]
        bql = np.concatenate(brows, axis=0)          # [384]
        wqkvT = np.ascontiguousarray(wql.T).astype(bf)
        bqkv2d = np.ascontiguousarray(bql.reshape(3, 128).T).astype(np.float32)
        in_maps.append({
            "xT": xT, "wqkvT": wqkvT, "bqkv2d": bqkv2d,
            "cosd": cosd, "sind": sind,
            "wprojT": wprojT, "bproj2d": bproj2d,
            "w1T": w1T, "w2Ts": w2Ts, "w3T": w3T,
            "b1h2d": b1h2d, "b2s2d": b2s2d, "b3row": b3row,
        })
    return in_maps


def kernel(x, w_qkv, b_qkv, w_proj, b_proj, w1, b1, w2, b2, w3, b3, cos, sin,
           _trace=False):
    from concourse import bass_utils

    b, t, c = x.shape
    assert (b, t, c) == (B, T, C)
    args = [np.asarray(a, dtype=np.float32) for a in
            (x, w_qkv, b_qkv, w_proj, b_proj, w1, b1, w2, b2, w3, b3,
             cos, sin)]
    nc = _get_program(b, t)
    in_maps = _prep_inputs(*args, b, t)
    res = bass_utils.run_bass_kernel_spmd(
        nc, in_maps, core_ids=list(range(NCORES)), trace=_trace)
    tok = b * t
    half = tok // (2 * NCORES)
    y = np.empty((tok, c), dtype=np.float32)
    for i in range(NCORES):
        yl = np.asarray(res.results[i]["y_loc"], np.float32)
        y[half * i:half * (i + 1), :] = yl[:half]
        y[t + half * i:t + half * (i + 1), :] = yl[half:]
    out = y.reshape(b, t, c)
    if _trace:
        return out, res
    return out


# revision 29
# speedup vs baseline: 1.0095x; 1.0095x over previous
"""Trainium2 Bass kernel for a dense transformer block (RoPE attention + SwiGLU).

Sharding (8 NeuronCores, Megatron-style):
  - QKV + attention: tensor-parallel over heads (2 heads/core, both batches).
  - Two AllToAlls (one per batch) reshard attention output from head-sharded
    to token-sharded (each core owns tpc/2 tokens of each batch).
  - proj + SwiGLU MLP: token-sharded, fully local, split into two halves so
    the first half overlaps batch-1 attention and the second hides A2A#1.
Engine queues are in-order, so the program interleaves QKV(b1) into the PE
gaps of attention(b0), and proj(half 0) into attention(b1).
All matmuls run in bf16 with fp32 PSUM accumulation.  silu is computed as
0.5*x*(1+tanh(x/2)) so the scalar engine stays on one activation table set
(exp+tanh); the 0.5 is folded into w2 on the host.
"""

import functools
import numpy as np
import ml_dtypes

B, T, C, H, D = 2, 2048, 1024, 16, 64
HID = 4 * C
NCORES = 8
HPC = H // NCORES          # heads per core
PACK_SCORES = True         # row-pack the two heads' score matmuls


def _build_program(b, t):
    import concourse.bacc as bacc
    import concourse.mybir as mybir
    import concourse.tile as tile
    import concourse.masks as masks
    from contextlib import ExitStack

    fp32 = mybir.dt.float32
    bf16 = mybir.dt.bfloat16
    Act = mybir.ActivationFunctionType
    Alu = mybir.AluOpType

    tok = b * t                    # all tokens (b-major)
    tpc = tok // NCORES            # tokens per core (proj/MLP/out)
    half = tpc // b                # tokens per core per batch
    m_qkv = 3 * HPC * D            # q, k, v local cols
    kt_tiles = t // 128            # 128-token key tiles per (b,h) unit
    qt_chunk = min(512, t)
    qcs = t // qt_chunk            # q chunks (= QKV n-chunks) per batch
    kt_pc = qt_chunk // 128        # kt tiles per QKV chunk
    ck = C // 128                  # C chunks (8)
    mh_tiles = HID // 128          # hidden chunks (32)
    hg = 8                         # hidden chunks per weight-stream group
    ngr = mh_tiles // hg           # weight groups per half
    tbs = min(128, half)           # token block size for w3
    ntb = half // tbs              # token blocks per half for w3
    scale = float(D) ** -0.5

    nc = bacc.Bacc("TRN2", target_bir_lowering=False, debug=False,
                   num_devices=NCORES)

    # ---- DRAM I/O ----
    xT_d = nc.dram_tensor("xT", [C, tok], bf16, kind="ExternalInput")
    wqkvT_d = nc.dram_tensor("wqkvT", [C, m_qkv], bf16, kind="ExternalInput")
    bqkv_d = nc.dram_tensor("bqkv2d", [128, 3], fp32, kind="ExternalInput")
    cos_d = nc.dram_tensor("cosd", [128, tok], bf16, kind="ExternalInput")
    sin_d = nc.dram_tensor("sind", [128, tok], bf16, kind="ExternalInput")
    wprojT_d = nc.dram_tensor("wprojT", [C, C], bf16, kind="ExternalInput")
    bproj_d = nc.dram_tensor("bproj2d", [128, ck], fp32, kind="ExternalInput")
    w1T_d = nc.dram_tensor("w1T", [C, HID], bf16, kind="ExternalInput")
    w2Ts_d = nc.dram_tensor("w2Ts", [C, HID], bf16, kind="ExternalInput")
    w3T_d = nc.dram_tensor("w3T", [HID, C], bf16, kind="ExternalInput")
    b1h_d = nc.dram_tensor("b1h2d", [128, mh_tiles], fp32, kind="ExternalInput")
    b2s_d = nc.dram_tensor("b2s2d", [128, mh_tiles], fp32, kind="ExternalInput")
    b3row_d = nc.dram_tensor("b3row", [1, C], bf16, kind="ExternalInput")
    y_d = nc.dram_tensor("y_loc", [tpc, C], fp32, kind="ExternalOutput")

    with tile.TileContext(nc) as tc:
        es = ExitStack()
        # ---- constants / biases (live whole kernel) ----
        consts = es.enter_context(tc.tile_pool(name="consts", bufs=1))
        ident = consts.tile([128, 128], bf16, name="ident")
        masks.make_identity(nc, ident[:])
        ones1 = consts.tile([65, 64], bf16, name="ones1")
        nc.vector.memset(ones1[:], 1.0)
        onescol = consts.tile([1, 128], bf16, name="onescol")
        nc.vector.memset(onescol[:], 1.0)
        lnscr = consts.tile([1, 8], fp32, name="lnscr")
        b3row = consts.tile([1, C], bf16, name="b3row")
        bqkv_sb = consts.tile([128, 3], fp32, name="bqkv_sb")
        bproj_sb = consts.tile([128, ck], fp32, name="bproj_sb")
        b1h_sb = consts.tile([128, mh_tiles], fp32, name="b1h_sb")
        b2s_sb = consts.tile([128, mh_tiles], fp32, name="b2s_sb")

        def emit_const_loads():
            nc.sync.dma_start(out=b3row[:], in_=b3row_d[:, :])
            nc.sync.dma_start(out=bqkv_sb[:], in_=bqkv_d[:, :])
            nc.sync.dma_start(out=bproj_sb[:], in_=bproj_d[:, :])
            nc.sync.dma_start(out=b1h_sb[:], in_=b1h_d[:, :])
            nc.sync.dma_start(out=b2s_sb[:], in_=b2s_d[:, :])

        # ---- DRAM bounce buffers for the two A2As ----
        dram = es.enter_context(tc.tile_pool(name="dramp", bufs=1,
                                             space="DRAM"))
        a2a_in = [dram.tile([NCORES * 128, half], bf16, name=f"a2a_in{i}")
                  for i in range(b)]
        a2a_out = [dram.tile([NCORES * 128, half], bf16, name=f"a2a_out{i}")
                   for i in range(b)]

        # ---- attention pools (live whole kernel; w3 reuses their PSUM) ----
        attn_pool = es.enter_context(tc.tile_pool(name="attn", bufs=1))
        qr = attn_pool.tile([128, tok], bf16, name="qr")
        kr = attn_pool.tile([128, tok], bf16, name="kr")
        vaug_cols = 65 * kt_tiles * b * HPC
        v_aug = attn_pool.tile([128, vaug_cols], bf16, name="v_aug")
        nc.vector.memset(v_aug[:], 1.0)
        outT0 = attn_pool.tile([64, tok], bf16, name="outT0")
        outT1 = attn_pool.tile([64, tok], bf16, name="outT1")
        ps_s = es.enter_context(
            tc.tile_pool(name="ps_s", bufs=2, space="PSUM"))
        ps_o = es.enter_context(
            tc.tile_pool(name="ps_o", bufs=4, space="PSUM"))
        expp = es.enter_context(tc.tile_pool(name="expp", bufs=3))
        smp = es.enter_context(tc.tile_pool(name="smp", bufs=2))

        # ---- QKV-phase pools (closed after QKV/rope/v of both batches) ----
        es_q = ExitStack()
        wq_pool = es_q.enter_context(tc.tile_pool(name="wq", bufs=1))
        rope_scr = es_q.enter_context(tc.tile_pool(name="ropescr", bufs=2))
        xt_pool = [None, None]
        qraw = [None, None]        # per-batch raw qkv (pool, tiles)

        wq_all = wq_pool.tile([128, ck * m_qkv], bf16, name="wq_all")
        for q in range(2):
            kc0 = q * (ck // 2)
            nc.sync.dma_start(
                out=wq_all[:, kc0 * m_qkv:(kc0 + ck // 2) * m_qkv].rearrange(
                    "p (k c) -> p k c", k=ck // 2),
                in_=wqkvT_d[128 * kc0:128 * (kc0 + ck // 2), :].rearrange(
                    "(k p) c -> p k c", p=128))
        wq_sb = [wq_all[:, kc * m_qkv:(kc + 1) * m_qkv] for kc in range(ck)]
        cos_sb = wq_pool.tile([128, tok], bf16, name="cos_sb")
        sin_sb = wq_pool.tile([128, tok], bf16, name="sin_sb")

        xt_sb = [[None] * ck, [None] * ck]

        def emit_x_loads(bi):
            pool = tc.alloc_tile_pool(name=f"xt{bi}", bufs=1)
            xt_pool[bi] = pool
            for kc in range(ck):
                xt_kc = pool.tile([128, t], bf16, name=f"xt{bi}_{kc}",
                                  tag=f"xt{kc}")
                for n in range(qcs):
                    nc.sync.dma_start(
                        out=xt_kc[:, n * qt_chunk:(n + 1) * qt_chunk],
                        in_=xT_d[128 * kc:128 * kc + 128,
                                 bi * t + n * qt_chunk:
                                 bi * t + (n + 1) * qt_chunk])
                xt_sb[bi][kc] = xt_kc

        def open_qraw(bi):
            pool = tc.alloc_tile_pool(name=f"qraw{bi}", bufs=1)
            tiles = [pool.tile([128, t], bf16, name=f"{nm}_{bi}")
                     for nm in ("q", "k", "v", "qsw", "ksw")]
            qraw[bi] = (pool, tiles)

        def emit_qkv_slice(bi, n, mi):
            """One [128 out-rows, qt_chunk tokens] slice of the QKV GEMM."""
            _, dest = qraw[bi]
            c0 = n * qt_chunk
            ps = ps_s.tile([128, qt_chunk], fp32, name=f"psq{bi}{n}{mi}",
                           tag="pss")
            for kc in range(ck):
                nc.tensor.matmul(
                    ps[:], wq_sb[kc][:, 128 * mi:128 * mi + 128],
                    xt_sb[bi][kc][:, c0:c0 + qt_chunk],
                    start=(kc == 0), stop=(kc == ck - 1))
            nc.vector.tensor_scalar_add(
                dest[mi][:, c0:c0 + qt_chunk], ps[:], bqkv_sb[:, mi:mi + 1])

        def emit_swaps(bi):
            """qsw/ksw = q/k with each head's two 32-row halves swapped."""
            _, d = qraw[bi]
            for src_t, dst_t in ((d[0], d[3]), (d[1], d[4])):
                for h in range(HPC):
                    for half32 in range(2):
                        s0 = 64 * h + 32 * half32
                        d0 = 64 * h + 32 * (1 - half32)
                        nc.sync.dma_start(
                            out=dst_t[d0:d0 + 32, :],
                            in_=src_t[s0:s0 + 32, :])

        def emit_rope(bi):
            _, d = qraw[bi]
            q_bf, k_bf, _, qsw_bf, ksw_bf = d
            cs = cos_sb[:, bi * t:bi * t + t]
            sn = sin_sb[:, bi * t:bi * t + t]
            for u_src, u_sw, u_dst in ((q_bf, qsw_bf, qr), (k_bf, ksw_bf, kr)):
                ta = rope_scr.tile([128, t], bf16, name="ta", tag="rs")
                nc.vector.tensor_mul(ta[:], u_src[:], cs)
                tb = rope_scr.tile([128, t], bf16, name="tb", tag="rs")
                nc.vector.tensor_mul(tb[:], u_sw[:], sn)
                nc.vector.tensor_add(u_dst[:, bi * t:bi * t + t], ta[:], tb[:])

        def emit_vtr(bi, kt):
            """Transpose v[:, kt] into v_aug for both heads."""
            _, d = qraw[bi]
            v_bf = d[2]
            for h in range(HPC):
                u = bi * HPC + h
                base = u * 65 * kt_tiles
                pst = ps_s.tile([128, 64], bf16, name=f"pst{u}_{kt}",
                                tag="pss")
                nc.tensor.transpose(
                    pst[:],
                    v_bf[64 * h:64 * h + 64, 128 * kt:128 * kt + 128],
                    ident[64 * h:64 * h + 64, 64 * h:64 * h + 64])
                nc.vector.tensor_copy(
                    v_aug[:, base + 65 * kt:base + 65 * kt + 64], pst[:])

        # ---------- deque-based interleave machinery ----------
        pending = []               # list of zero-arg emitters

        def pump(k):
            for _ in range(min(k, len(pending))):
                pending.pop(0)()

        def emit_unit_pair(bi, qlist):
            """Attention for both heads of (batch bi, q-chunks qlist).

            Both heads' scores for one kt land in one 2-bank psum tile
            (row-packed matmuls writing the two column halves), so a single
            FD=2*qt_chunk exp evacuates the pair.  Two q-chunks are kept in
            flight to hide the PE<->ACT semaphore latency."""
            q0 = {qc: bi * t + qc * qt_chunk for qc in qlist}
            pso = {(qc, h): ps_o.tile([65, qt_chunk], fp32,
                                      name=f"pso{bi}{qc}{h}", tag="pso")
                   for qc in qlist for h in range(HPC)}
            # dummy Ln keeps this group's exps on the ln+exp table set
            nc.scalar.activation(lnscr[:, :], ones1[0:1, 0:8], Act.Ln)
            for kt in range(kt_tiles):
                pss = {}
                for qc in qlist:
                    p = ps_s.tile([128, 2 * qt_chunk], fp32,
                                  name=f"pss{bi}{qc}{kt}", tag="pss")
                    for h in range(HPC):
                        tp = (64 * h, 0) if PACK_SCORES else None
                        nc.tensor.matmul(
                            p[:, h * qt_chunk:(h + 1) * qt_chunk],
                            kr[64 * h:64 * h + 64,
                               bi * t + 128 * kt:bi * t + 128 * kt + 128],
                            qr[64 * h:64 * h + 64,
                               q0[qc]:q0[qc] + qt_chunk],
                            start=True, stop=True, tile_position=tp)
                    pss[qc] = p
                exps = {}
                for qc in qlist:
                    e = expp.tile([128, 2 * qt_chunk], bf16,
                                  name=f"exp{bi}{qc}{kt}", tag="e")
                    nc.scalar.activation(e[:], pss[qc][:], Act.Exp,
                                         scale=scale)
                    exps[qc] = e
                for qc in qlist:
                    for h in range(HPC):
                        u = bi * HPC + h
                        vbase = u * 65 * kt_tiles
                        nc.tensor.matmul(
                            pso[(qc, h)][:],
                            v_aug[:, vbase + 65 * kt:vbase + 65 * kt + 65],
                            exps[qc][:, h * qt_chunk:(h + 1) * qt_chunk],
                            start=(kt == 0), stop=(kt == kt_tiles - 1))
                if kt % 4 == 3:
                    pump(3)
            # normalization: 1/s = exp(-ln(s)) on ACT, broadcast via
            # matmul, final mul on DVE
            for qc in qlist:
                for h in range(HPC):
                    lnv = smp.tile([65, qt_chunk], fp32,
                                   name=f"ln{bi}{qc}{h}", tag="lnv", bufs=2)
                    nc.scalar.activation(lnv[64:65, :],
                                         pso[(qc, h)][64:65, :], Act.Ln)
                    rb = smp.tile([65, qt_chunk], bf16,
                                  name=f"rb{bi}{qc}{h}", tag="rb", bufs=2)
                    nc.scalar.activation(rb[64:65, :], lnv[64:65, :],
                                         Act.Exp, scale=-1.0)
                    psb = ps_s.tile([64, qt_chunk], fp32,
                                    name=f"psb{bi}{qc}{h}", tag="pss")
                    nc.tensor.matmul(psb[:], ones1[64:65, :], rb[64:65, :],
                                     start=True, stop=True)
                    bc = smp.tile([64, qt_chunk], fp32,
                                  name=f"bc{bi}{qc}{h}", tag="bc", bufs=2)
                    nc.vector.tensor_copy(bc[:], psb[:])
                    out_h = outT0 if h == 0 else outT1
                    nc.vector.tensor_mul(
                        out_h[:, q0[qc]:q0[qc] + qt_chunk],
                        pso[(qc, h)][0:64, :], bc[:])

        def emit_a2a_in(bi, qc):
            lo = qc * qt_chunk
            hi = lo + qt_chunk
            for c in range(NCORES):
                s0 = c * half
                if s0 < lo or s0 >= hi:
                    continue
                for h, out_h in ((0, outT0), (1, outT1)):
                    nc.sync.dma_start(
                        out=a2a_in[bi][128 * c + 64 * h:
                                       128 * c + 64 * h + 64, :],
                        in_=out_h[:, bi * t + s0:bi * t + s0 + half])

        # ================= program =================
        open_qraw(0)
        emit_x_loads(0)
        emit_const_loads()
        for n in range(qcs):
            for mi in range(3):
                emit_qkv_slice(0, n, mi)
        xt_pool[0].release()
        nc.sync.dma_start(out=cos_sb[:], in_=cos_d[:, :])
        nc.sync.dma_start(out=sin_sb[:], in_=sin_d[:, :])
        emit_swaps(0)
        emit_rope(0)
        for kt in range(kt_tiles):
            emit_vtr(0, kt)
        qraw[0][0].release()

        # queue QKV b1 + v transposes b1 + rope b1 as pump items
        open_qraw(1)
        emit_x_loads(1)
        vtr_items = []
        for n in range(qcs):
            for mi in range(3):
                pending.append(
                    (lambda n=n, mi=mi: emit_qkv_slice(1, n, mi)))
            for kt in range(n * kt_pc, (n + 1) * kt_pc):
                vtr_items.append((lambda kt=kt: emit_vtr(1, kt)))
        pending.append(lambda: emit_swaps(1))
        pending.append(lambda: emit_rope(1))
        pending.extend(vtr_items)

        # ---- attention batch 0, QKV b1 interleaved ----
        for qp in range(0, qcs, 2):
            qlist = tuple(range(qp, min(qp + 2, qcs)))
            emit_unit_pair(0, qlist)
            for qc in qlist:
                emit_a2a_in(0, qc)
            pump(6)
        pump(len(pending))
        # QKV pools no longer needed; swap PSUM/SBUF budget to MLP pools
        xt_pool[1].release()
        qraw[1][0].release()
        es_q.close()
        nc.gpsimd.collective_compute(
            "AllToAll", Alu.bypass,
            replica_groups=[list(range(NCORES))],
            ins=[a2a_in[0][:]], outs=[a2a_out[0][:]])

        wg_pool = es.enter_context(tc.tile_pool(name="wg", bufs=2))
        mlp_scr = es.enter_context(tc.tile_pool(name="mlp_scr", bufs=2))
        wp_pool = es.enter_context(tc.tile_pool(name="wpp", bufs=1))
        of_pool = es.enter_context(tc.tile_pool(name="ofp", bufs=1))
        mlp_sb = es.enter_context(tc.tile_pool(name="mlp_sb", bufs=1))
        aT = [mlp_sb.tile([128, ck * half], bf16, name=f"aT{i}")
              for i in range(b)]
        hT = [mlp_sb.tile([128, mh_tiles * half], bf16, name=f"hT{i}")
              for i in range(b)]

        wp_all = wp_pool.tile([128, ck * C], bf16, name="wp_all")
        nc.sync.dma_start(
            out=wp_all[:].rearrange("p (k c) -> p k c", k=ck),
            in_=wprojT_d[:, :].rearrange("(k p) c -> p k c", p=128))
        wp_sb = [wp_all[:, kc * C:(kc + 1) * C] for kc in range(ck)]

        of_sb = [[None] * ck, [None] * ck]

        def emit_of_loads(hf):
            of_all = of_pool.tile([128, ck * half], bf16, name=f"of{hf}",
                                  tag=f"of{hf}")
            nc.sync.dma_start(
                out=of_all[:].rearrange("p (k c) -> p k c", k=ck),
                in_=a2a_out[hf][:, :].rearrange("(k p) c -> p k c", p=128))
            for kc in range(ck):
                of_sb[hf][kc] = of_all[:, kc * half:(kc + 1) * half]

        def emit_proj_slice(hf, m):
            psa = ps_o.tile([128, half], fp32, name=f"psa{hf}{m}", tag="pso")
            for kc in range(ck):
                nc.tensor.matmul(psa[:],
                                 wp_sb[kc][:, 128 * m:128 * m + 128],
                                 of_sb[hf][kc][:],
                                 start=(kc == 0), stop=(kc == ck - 1))
            nc.vector.tensor_scalar_add(
                aT[hf][:, half * m:half * (m + 1)], psa[:],
                bproj_sb[:, m:m + 1])

        wg_sb = {}

        def emit_wg_loads(hf, g):
            gw = hg * 128
            for wname, wd in (("w1", w1T_d), ("w2", w2Ts_d)):
                wt = wg_pool.tile([128, ck * gw], bf16,
                                  name=f"{wname}g{hf}{g}", tag=f"{wname}g")
                for q in range(2):
                    kc0 = q * (ck // 2)
                    nc.sync.dma_start(
                        out=wt[:, kc0 * gw:(kc0 + ck // 2) * gw].rearrange(
                            "p (k c) -> p k c", k=ck // 2),
                        in_=wd[128 * kc0:128 * (kc0 + ck // 2),
                               gw * g:gw * (g + 1)].rearrange(
                            "(k p) c -> p k c", p=128))
                wg_sb[(hf, wname, g)] = [wt[:, kc * gw:(kc + 1) * gw]
                                         for kc in range(ck)]

        def emit_mlp_tile(hf, mh):
            g, ml = mh // hg, mh % hg
            w1g = wg_sb[(hf, "w1", g)]
            w2g = wg_sb[(hf, "w2", g)]
            ps1 = ps_o.tile([128, half], fp32, name=f"ps1_{hf}{mh}",
                            tag="pso")
            for kc in range(ck):
                nc.tensor.matmul(ps1[:],
                                 w1g[kc][:, 128 * ml:128 * ml + 128],
                                 aT[hf][:, half * kc:half * (kc + 1)],
                                 start=(kc == 0), stop=(kc == ck - 1))
            ps2 = ps_o.tile([128, half], fp32, name=f"ps2_{hf}{mh}",
                            tag="pso")
            for kc in range(ck):
                nc.tensor.matmul(ps2[:],
                                 w2g[kc][:, 128 * ml:128 * ml + 128],
                                 aT[hf][:, half * kc:half * (kc + 1)],
                                 start=(kc == 0), stop=(kc == ck - 1))
            th = mlp_scr.tile([128, half], fp32, name=f"th{hf}{mh}", tag="th")
            nc.scalar.activation(th[:], ps1[:], Act.Tanh, scale=0.5,
                                 bias=b1h_sb[:, mh:mh + 1])
            uu = mlp_scr.tile([128, half], fp32, name=f"u{hf}{mh}", tag="u")
            nc.vector.scalar_tensor_tensor(uu[:], th[:], 1.0, ps1[:],
                                           op0=Alu.add, op1=Alu.mult)
            nc.vector.scalar_tensor_tensor(
                hT[hf][:, half * mh:half * (mh + 1)], ps2[:],
                b2s_sb[:, mh:mh + 1], uu[:], op0=Alu.add, op1=Alu.mult)


        def emit_w3_group(hf, yps, khp):
            w3k = w3_pool.tile([128, 2 * C], bf16, name=f"w3k{hf}{khp}",
                               tag="w3k")
            nc.sync.dma_start(
                out=w3k[:].rearrange("p (k c) -> p k c", k=2),
                in_=w3T_d[256 * khp:256 * (khp + 1), :].rearrange(
                    "(k p) c -> p k c", p=128))
            for j in range(2):
                kh = 2 * khp + j
                for tb in range(ntb):
                    hslice = hT[hf][:, half * kh + tbs * tb:
                                    half * kh + tbs * (tb + 1)]
                    for ch in range(C // 512):
                        nc.tensor.matmul(
                            yps[tb][:, 512 * ch:512 * (ch + 1)],
                            hslice, w3k[:, C * j + 512 * ch:
                                        C * j + 512 * (ch + 1)],
                            start=(kh == 0), stop=False)

        def emit_w3_tail(hf, yps):
            for tb in range(ntb):
                ysb = ysb_pool.tile([tbs, C], fp32, name=f"ysb{hf}{tb}",
                                    tag="ysb")
                for ch in range(C // 512):
                    nc.tensor.matmul(
                        yps[tb][:, 512 * ch:512 * (ch + 1)],
                        onescol[0:1, 0:tbs],
                        b3row[0:1, 512 * ch:512 * (ch + 1)],
                        start=False, stop=True)
                    nc.vector.tensor_copy(ysb[:, 512 * ch:512 * (ch + 1)],
                                          yps[tb][:, 512 * ch:512 * (ch + 1)])
                nc.sync.dma_start(
                    out=y_d[half * hf + tbs * tb:half * hf + tbs * (tb + 1),
                            :],
                    in_=ysb[:])

        def emit_mlp_half(hf):
            """MLP tiles with the w3 GEMM interleaved as hT tiles appear."""
            yps = [ps_s.tile([tbs, C], fp32, name=f"yps{hf}{tb}",
                             tag="pss") for tb in range(ntb)]
            for mh in range(mh_tiles):
                if mh % hg == 0:
                    gi = mh // hg
                    if gi + 1 < ngr:
                        emit_wg_loads(hf, gi + 1)
                    elif hf == 0:
                        emit_wg_loads(1, 0)
                emit_mlp_tile(hf, mh)
                if mh % 2 == 1:
                    emit_w3_group(hf, yps, mh // 2)
            emit_w3_tail(hf, yps)

        # ---- attention batch 1, of0+proj half 0 pumped in the last pair ----
        emit_wg_loads(0, 0)
        for qp in range(0, qcs, 2):
            qlist = tuple(range(qp, min(qp + 2, qcs)))
            if qcs > 1 and qp + 2 >= qcs:
                pending.append(lambda: emit_of_loads(0))
                for m in range(ck):
                    pending.append((lambda m=m: emit_proj_slice(0, m)))
            emit_unit_pair(1, qlist)
            for qc in qlist:
                emit_a2a_in(1, qc)
            pump(4)
        pump(len(pending))

        nc.gpsimd.collective_compute(
            "AllToAll", Alu.bypass,
            replica_groups=[list(range(NCORES))],
            ins=[a2a_in[1][:]], outs=[a2a_out[1][:]])
        if qcs == 1:   # small-t sim path: emit of0+proj directly
            emit_of_loads(0)
            for m in range(ck):
                emit_proj_slice(0, m)

        # ---- MLP+w3 half 0 (hides A2A#1), then half 1 ----
        w3_pool = es.enter_context(tc.tile_pool(name="w3p", bufs=3))
        ysb_pool = es.enter_context(tc.tile_pool(name="ysb", bufs=1))
        emit_of_loads(1)
        emit_mlp_half(0)
        for m in range(ck):
            emit_proj_slice(1, m)
        emit_mlp_half(1)
        es.close()

    nc.compile()
    return nc


@functools.lru_cache(maxsize=2)
def _get_program(b, t):
    return _build_program(b, t)


def _prep_inputs(x, w_qkv, b_qkv, w_proj, b_proj, w1, b1, w2, b2, w3, b3,
                 cos, sin, b, t):
    """Build per-core in_maps (host-side sharding / transposes / casts)."""
    bf = ml_dtypes.bfloat16
    tok = b * t
    ck = C // 128
    mh_tiles = HID // 128

    xT = np.ascontiguousarray(x.reshape(tok, C).T).astype(bf)
    # RoPE tables tiled to [128, tok]: rows = 4x the 32 freq rows,
    # cols = b-major tokens.
    cosd = np.tile(cos.T, (4, b)).astype(bf)
    sind = np.tile(np.concatenate([-sin.T, sin.T], axis=0), (2, b)).astype(bf)
    wprojT = np.ascontiguousarray(w_proj.T).astype(bf)
    bproj2d = np.ascontiguousarray(b_proj.reshape(ck, 128).T).astype(np.float32)
    w1T = np.ascontiguousarray(w1.T).astype(bf)
    w2Ts = np.ascontiguousarray(0.5 * w2.T).astype(bf)
    w3T = np.ascontiguousarray(w3.T).astype(bf)
    b1h2d = np.ascontiguousarray(
        (0.5 * b1).reshape(mh_tiles, 128).T).astype(np.float32)
    b2s2d = np.ascontiguousarray(
        (0.5 * b2).reshape(mh_tiles, 128).T).astype(np.float32)
    b3row = np.ascontiguousarray(b3.reshape(1, C)).astype(bf)

    # even/odd RoPE permutation within each head's 64 dims, plus the
    # half-swapped copy used to make RoPE partition-aligned on device
    perm = np.concatenate([np.arange(0, D, 2), np.arange(1, D, 2)])
    perm_sw = np.concatenate([np.arange(1, D, 2), np.arange(0, D, 2)])

    in_maps = []
    for c in range(NCORES):
        rows = []
        brows = []
        secperm = [(0, perm), (1, perm), (2, np.arange(D))]
        for sec, p in secperm:        # q, k, v
            for hh in range(HPC):
                h = HPC * c + hh
                idx = sec * H * D + h * D + p
                rows.append(w_qkv[idx, :])
                brows.append(b_qkv[idx])
        wql = np.concatenate(rows, axis=0)           # [384, C]
        bql = np.concatenate(brows, axis=0)          # [384]
        wqkvT = np.ascontiguousarray(wql.T).astype(bf)
        bqkv2d = np.ascontiguousarray(bql.reshape(3, 128).T).astype(np.float32)
        in_maps.append({
            "xT": xT, "wqkvT": wqkvT, "bqkv2d": bqkv2d,
            "cosd": cosd, "sind": sind,
            "wprojT": wprojT, "bproj2d": bproj2d,
            "w1T": w1T, "w2Ts": w2Ts, "w3T": w3T,
            "b1h2d": b1h2d, "b2s2d": b2s2d, "b3row": b3row,
        })
    return in_maps


def kernel(x, w_qkv, b_qkv, w_proj, b_proj, w1, b1, w2, b2, w3, b3, cos, sin,
           _trace=False):
    from concourse import bass_utils

    b, t, c = x.shape
    assert (b, t, c) == (B, T, C)
    args = [np.asarray(a, dtype=np.float32) for a in
            (x, w_qkv, b_qkv, w_proj, b_proj, w1, b1, w2, b2, w3, b3,
             cos, sin)]
    nc = _get_program(b, t)
    in_maps = _prep_inputs(*args, b, t)
    res = bass_utils.run_bass_kernel_spmd(
        nc, in_maps, core_ids=list(range(NCORES)), trace=_trace)
    tok = b * t
    half = tok // (2 * NCORES)
    y = np.empty((tok, c), dtype=np.float32)
    for i in range(NCORES):
        yl = np.asarray(res.results[i]["y_loc"], np.float32)
        y[half * i:half * (i + 1), :] = yl[:half]
        y[t + half * i:t + half * (i + 1), :] = yl[half:]
    out = y.reshape(b, t, c)
    if _trace:
        return out, res
    return out
